# revision 1
# baseline (speedup 1.0000x reference)
"""EntityMultiAttnQMixer kernel — 8-way data-parallel over NeuronCores.

Shards the flattened batch*time dimension (BS = 64*256 = 16384) across the
8 cores; the ~100KB of params are replicated. Each shard runs the fused
embed -> 4x tiny multi-head attention -> hypernet mixing chain and returns
its [BS/8, 1] slice of q_tot.
"""

import numpy as np
import jax
import jax.numpy as jnp
from functools import partial

B, S, T, E = 64, 256, 32, 64
D, H, A = 64, 4, 16
NCORES = 8
BS = B * S


def _mha(x, key_mask, p):
    # x: [N, T, D]; key_mask: [N, T] bool, True = masked key
    n, t, d = x.shape
    hd = d // H

    def proj(w, b):
        return (x @ w + b).reshape(n, t, H, hd).transpose(0, 2, 1, 3)

    q, k, v = proj(*p['q']), proj(*p['k']), proj(*p['v'])
    logits = jnp.einsum('bhqd,bhkd->bhqk', q, k) / jnp.sqrt(jnp.float32(hd))
    logits = jnp.where(key_mask[:, None, None, :], -1e9, logits)
    attn = jax.nn.softmax(logits, axis=-1)
    out = jnp.einsum('bhqk,bhkd->bhqd', attn, v).transpose(0, 2, 1, 3).reshape(n, t, d)
    wo, bo = p['o']
    return out @ wo + bo


def _shard_fn(agent_qs, entities, entity_mask, params):
    # agent_qs: [N, A]; entities: [N, T, E]; entity_mask: [N, T]
    emask = entity_mask
    amask = emask[:, :A]
    qs = agent_qs[:, None, :]                       # [N, 1, A]

    def z(nname):
        ew, eb = params[f'emb_{nname}']
        x = jax.nn.relu(entities @ ew + eb)                  # [N, T, D]
        y = _mha(x, emask, params[f'attn_{nname}'])[:, :A]   # [N, A, D]
        y = jnp.where(amask[:, :, None], 0.0, y)
        hw, hb = params[f'hyper_{nname}']
        return y @ hw + hb                                   # [N, A, D]

    w_1 = jax.nn.softmax(z('w1'), axis=-1)
    b_1 = jnp.where(amask[:, :, None], 0.0, z('b1')).mean(1, keepdims=True)
    h = jax.nn.elu(qs @ w_1 + b_1)
    w_2 = jnp.where(amask[:, :, None], 0.0,
                    jax.nn.softmax(z('w2'), axis=-1)).mean(1, keepdims=True)
    b_2 = jnp.where(amask[:, :, None], 0.0, z('b2')).mean((1, 2), keepdims=True)
    q_tot = h @ w_2.transpose(0, 2, 1) + b_2
    return q_tot[:, 0, :]                                    # [N, 1]


_pmapped = None


def _get_pmapped():
    global _pmapped
    if _pmapped is None:
        _pmapped = jax.pmap(_shard_fn, in_axes=(0, 0, 0, None),
                            devices=jax.devices()[:NCORES])
    return _pmapped


def kernel(agent_qs, entities, entity_mask, params):
    agent_qs = np.asarray(agent_qs, dtype=np.float32).reshape(NCORES, BS // NCORES, A)
    entities = np.asarray(entities, dtype=np.float32).reshape(NCORES, BS // NCORES, T, E)
    entity_mask = np.asarray(entity_mask).reshape(NCORES, BS // NCORES, T)
    params = jax.tree_util.tree_map(lambda a: jnp.asarray(a, jnp.float32), params)
    out = _get_pmapped()(agent_qs, entities, entity_mask, params)
    out = np.asarray(out).reshape(B, S, 1).astype(np.float32)
    return out


# revision 2
# speedup vs baseline: 1.2427x; 1.2427x over previous
"""EntityMultiAttnQMixer kernel — 8-way data-parallel over NeuronCores.

Shards the flattened batch*time dimension (BS = 64*256 = 16384) across the
8 cores; the ~100KB of params are replicated. The 4 hypernet branches
(w1/b1/w2/b2) are stacked and vmapped so each matmul is 4x larger, queries
are only computed for the A=16 agent slots actually used, and head
splitting avoids data transposes (einsum on [N,T,H,hd] directly).
"""

import numpy as np
import jax
import jax.numpy as jnp

B, S, T, E = 64, 256, 32, 64
D, H, A = 64, 4, 16
HD = D // H
NCORES = 8
BS = B * S
NETS = ('w1', 'b1', 'w2', 'b2')


def _shard_fn(agent_qs, entities, entity_mask, P):
    # agent_qs: [N, A]; entities: [N, T, E]; entity_mask: [N, T] bool
    # P: dict of per-net stacked params, leading axis 4 in NETS order.
    n = entities.shape[0]
    emask = entity_mask
    amask = emask[:, :A]                                   # [N, A]
    kmask = emask[:, None, None, :]                        # [N, 1, 1, T]
    qs = agent_qs[:, None, :]                              # [N, 1, A]

    def z_one(ew, eb, qw, qb, kw, kb, vw, vb, ow, ob, hw, hb):
        x = jax.nn.relu(entities @ ew + eb)                # [N, T, D]
        # queries only needed for the first A tokens
        q = (x[:, :A] @ qw + qb).reshape(n, A, H, HD)
        k = (x @ kw + kb).reshape(n, T, H, HD)
        v = (x @ vw + vb).reshape(n, T, H, HD)
        logits = jnp.einsum('bqhc,bkhc->bhqk', q, k) / jnp.sqrt(jnp.float32(HD))
        logits = jnp.where(kmask, -1e9, logits)
        attn = jax.nn.softmax(logits, axis=-1)
        out = jnp.einsum('bhqk,bkhc->bqhc', attn, v).reshape(n, A, D)
        y = out @ ow + ob                                  # [N, A, D]
        y = jnp.where(amask[:, :, None], 0.0, y)
        return y @ hw + hb                                 # [N, A, D]

    z_all = jax.vmap(z_one)(
        P['emb_w'], P['emb_b'], P['q_w'], P['q_b'], P['k_w'], P['k_b'],
        P['v_w'], P['v_b'], P['o_w'], P['o_b'], P['hyper_w'], P['hyper_b'])
    z_w1, z_b1, z_w2, z_b2 = (z_all[i] for i in range(4))

    w_1 = jax.nn.softmax(z_w1, axis=-1)                    # [N, A, D]
    b_1 = jnp.where(amask[:, :, None], 0.0, z_b1).mean(1, keepdims=True)
    h = jax.nn.elu(qs @ w_1 + b_1)                         # [N, 1, D]
    w_2 = jnp.where(amask[:, :, None], 0.0,
                    jax.nn.softmax(z_w2, axis=-1)).mean(1, keepdims=True)
    b_2 = jnp.where(amask[:, :, None], 0.0, z_b2).mean((1, 2), keepdims=True)
    q_tot = h @ w_2.transpose(0, 2, 1) + b_2               # [N, 1, 1]
    return q_tot[:, 0, :]                                  # [N, 1]


_pmapped = None


def _get_pmapped():
    global _pmapped
    if _pmapped is None:
        _pmapped = jax.pmap(_shard_fn, in_axes=(0, 0, 0, None),
                            devices=jax.devices()[:NCORES])
    return _pmapped


def _stack_params(params):
    f32 = lambda a: np.asarray(a, np.float32)
    P = {}
    P['emb_w'] = np.stack([f32(params[f'emb_{n}'][0]) for n in NETS])
    P['emb_b'] = np.stack([f32(params[f'emb_{n}'][1]) for n in NETS])
    for proj in ('q', 'k', 'v', 'o'):
        P[f'{proj}_w'] = np.stack([f32(params[f'attn_{n}'][proj][0]) for n in NETS])
        P[f'{proj}_b'] = np.stack([f32(params[f'attn_{n}'][proj][1]) for n in NETS])
    P['hyper_w'] = np.stack([f32(params[f'hyper_{n}'][0]) for n in NETS])
    P['hyper_b'] = np.stack([f32(params[f'hyper_{n}'][1]) for n in NETS])
    return {k: jnp.asarray(v) for k, v in P.items()}


def kernel(agent_qs, entities, entity_mask, params):
    agent_qs = np.asarray(agent_qs, dtype=np.float32).reshape(NCORES, BS // NCORES, A)
    entities = np.asarray(entities, dtype=np.float32).reshape(NCORES, BS // NCORES, T, E)
    entity_mask = np.asarray(entity_mask).reshape(NCORES, BS // NCORES, T)
    P = _stack_params(params)
    out = _get_pmapped()(agent_qs, entities, entity_mask, P)
    out = np.asarray(out).reshape(B, S, 1).astype(np.float32)
    return out
